# revision 57
# baseline (speedup 1.0000x reference)
"""AxialAttention Trainium2 kernel (8-core data-parallel over batch), v2.

Per image: qkv = x @ qkv_w + alpha*img; per head (16, dh=64) axial-roped
q,k; scores along W per row (no softmax); v row-summed; GroupNorm per
(b, head); output projection.

Algebraic simplifications (exact):
  - per-head gamma scale on k is removed by GroupNorm -> dropped.
  - height-half rope rotations cancel in q.k (same row, orthogonal) ->
    rope only on width-half features (32 of 64 per head).
  - v is only needed row-summed: vsum = (A @ x) @ Wv + (A @ (alpha*img))
    -> per-token v projection skipped entirely.

v2/v3 changes vs v1:
  - bf16 everywhere on-chip except PSUM accumulation and the final
    fp32 output (4x matmul throughput vs fp32, 2x DVE, half DMA).
  - single pass over x (no separate phase 0).
  - vsum computed fully on the host (0.2% of FLOPs): vsum = (A@x)@Wv
    + alpha*A@img, shipped per image duplicated at partitions 0:28
    and 32:60.
  - rope is linear, so alpha*img is roped on the host and shipped
    feature-major; it is added AFTER the PE transposes, fused into the
    psum->sbuf copy (tensor_tensor add). The device ropes only x@Wqk.
  - attention/groupnorm at partitions 0:28 (0:64) for every image;
    head-pair-wide (128-partition) stats/normalize passes.
  - work spread across PE/DVE/ACT/GPSIMD.
"""

import math
import sys

import numpy as np

for _p in ("/opt/trn_rl_repo", "/root/.axon_site/_ro/trn_rl_repo"):
    if _p not in sys.path:
        sys.path.append(_p)

import concourse.bacc as bacc
import concourse.mybir as mybir
from concourse import bass_isa, tile
from concourse.bass_utils import run_bass_kernel_spmd

F32 = mybir.dt.float32
BF16 = mybir.dt.bfloat16
ALU = mybir.AluOpType
ACTF = mybir.ActivationFunctionType

HEADS = 16
DH = 64
H = W = 28
HID = 1024
B_FULL = 32
N_CORES = 8
B_CORE = B_FULL // N_CORES          # 4 images per core
TOK = B_CORE * H * W                # 3136 tokens per core
TT = 112                            # tokens per tile (4 rows)
TPI = H * W                         # 784 tokens per image
JPI = TPI // TT                     # 7 tiles per image
NTILES = TOK // TT                  # 28
ALPHA = 1.0 - math.tanh(math.pi * 6.0 / 12.0)
EPS = 1e-5
NGRP = float(H * W * DH)

_CACHE = {}


def _build_program(gn_w, gn_b, stage="full"):
    nc = bacc.Bacc("TRN2", target_bir_lowering=False, debug=False,
                   num_devices=N_CORES)

    x_d = nc.dram_tensor("x", [TOK, HID], BF16, kind="ExternalInput").ap()
    # [28 tiles][128 part][8 chunks x 112 tok], host pre-arranged
    imgT_d = nc.dram_tensor("imgT", [NTILES * 128, 8 * TT], BF16,
                            kind="ExternalInput").ap()
    vsum_d = nc.dram_tensor("vsum", [B_CORE * 64, HID], BF16,
                            kind="ExternalInput").ap()
    wqk_d = nc.dram_tensor("wqk", [HID, 2 * HID], BF16, kind="ExternalInput").ap()
    wo_d = nc.dram_tensor("wo", [HID, HID], BF16, kind="ExternalInput").ap()
    idn_d = nc.dram_tensor("idn", [128, 128], BF16, kind="ExternalInput").ap()
    ct_d = nc.dram_tensor("ctab", [TT, 512], BF16, kind="ExternalInput").ap()
    st_d = nc.dram_tensor("stab", [TT, 512], BF16, kind="ExternalInput").ap()
    y_d = nc.dram_tensor("y", [TOK, HID], F32, kind="ExternalOutput").ap()

    from contextlib import ExitStack
    with ExitStack() as ctx:
        tc = ctx.enter_context(tile.TileContext(nc))
        constp = ctx.enter_context(tc.tile_pool(name="const", bufs=1))
        wqkp = ctx.enter_context(tc.tile_pool(name="wqk", bufs=1))
        wop = ctx.enter_context(tc.tile_pool(name="wo", bufs=1))
        xinp = ctx.enter_context(tc.tile_pool(name="xin", bufs=3))
        itTp = ctx.enter_context(tc.tile_pool(name="itT", bufs=3))
        xtsp = ctx.enter_context(tc.tile_pool(name="xts", bufs=3))
        qkcp = ctx.enter_context(tc.tile_pool(name="qkc", bufs=4))
        rtp = ctx.enter_context(tc.tile_pool(name="rt", bufs=3))
        qfkp = ctx.enter_context(tc.tile_pool(name="qfk", bufs=2))
        ofp = ctx.enter_context(tc.tile_pool(name="of", bufs=2))
        of2p = ctx.enter_context(tc.tile_pool(name="of2", bufs=2))
        vsump = ctx.enter_context(tc.tile_pool(name="vsum", bufs=2))
        stsbp = ctx.enter_context(tc.tile_pool(name="stsb", bufs=3))
        sqdp = ctx.enter_context(tc.tile_pool(name="sqd", bufs=1))
        statp = ctx.enter_context(tc.tile_pool(name="stat", bufs=2))
        youtp = ctx.enter_context(tc.tile_pool(name="yout", bufs=2))
        # PSUM: pq(3) + ptr(1) + st(2) + ot(2) = 8 banks
        pqp = ctx.enter_context(tc.tile_pool(name="pq", bufs=3, space="PSUM"))
        ptrp = ctx.enter_context(tc.tile_pool(name="ptr", bufs=1, space="PSUM"))
        pstp = ctx.enter_context(tc.tile_pool(name="pst", bufs=1, space="PSUM"))
        potp = ctx.enter_context(tc.tile_pool(name="pot", bufs=1, space="PSUM"))

        idn = constp.tile([128, 128], BF16, tag="idn")
        nc.sync.dma_start(idn[:], idn_d[:])
        ct = constp.tile([TT, 512], BF16, tag="ct")
        nc.sync.dma_start(ct[:], ct_d[:])
        st = constp.tile([TT, 512], BF16, tag="st")
        nc.sync.dma_start(st[:], st_d[:])
        epsb = constp.tile([128, 1], F32, tag="epsb")
        nc.gpsimd.memset(epsb[:], EPS)
        # gw2/gb2 [128, 8]: col p = gamma/beta of head 2p on rows 0:64,
        # head 2p+1 on rows 64:128
        gw2 = constp.tile([128, 8], F32, tag="gw2")
        gb2 = constp.tile([128, 8], F32, tag="gb2")
        for p in range(8):
            nc.gpsimd.memset(gw2[0:64, p:p + 1], float(gn_w[2 * p]))
            nc.gpsimd.memset(gw2[64:128, p:p + 1], float(gn_w[2 * p + 1]))
            nc.gpsimd.memset(gb2[0:64, p:p + 1], float(gn_b[2 * p]))
            nc.gpsimd.memset(gb2[64:128, p:p + 1], float(gn_b[2 * p + 1]))

        wqk_sb = []
        for k in range(8):
            t = wqkp.tile([128, 2 * HID], BF16, tag=f"wqk{k}", name=f"wqk_sb{k}")
            nc.sync.dma_start(t[:], wqk_d[128 * k:128 * (k + 1), :])
            wqk_sb.append(t)
        wo_sb = []
        of2_l = [None] * B_CORE

        # program-lifetime scores psum tiles; gap partitions zeroed once
        stp_ab = [pstp.tile([64, 448], F32, tag=f"st{h}", name=f"stp{h}")
                  for h in range(2)]
        for h in range(2):
            nc.vector.memset(stp_ab[h][:], 0.0)

        def _outproj(bb):
            for j in range(JPI):
                for nn in range(2):
                    yp = pqp.tile([TT, 512], F32, tag="pq",
                                  name=f"yp{bb}_{j}_{nn}")
                    for k in range(8):
                        nc.tensor.matmul(
                            yp[:],
                            of2_l[bb][:, TPI * k + TT * j:TPI * k + TT * (j + 1)],
                            wo_sb[k][:, 512 * nn:512 * (nn + 1)],
                            start=(k == 0), stop=(k == 7))
                    y_sb = youtp.tile([TT, 512], F32, tag="y_sb")
                    nc.scalar.activation(y_sb[:], yp[:], ACTF.Copy)
                    nc.sync.dma_start(
                        y_d[TPI * bb + TT * j:TPI * bb + TT * (j + 1),
                            512 * nn:512 * (nn + 1)], y_sb[:])

        for b in range(B_CORE):
            # ---------------- tile loop: proj + rope ----------------
            qfk = qfkp.tile([128, 16 * TPI], BF16, tag="qfk", name=f"qfk{b}")
            qfkv = qfk[:].rearrange("p (cg t) -> p cg t", t=TPI)
            vsum = vsump.tile([64, HID], BF16, tag="vsum", name=f"vsum{b}")
            nc.sync.dma_start(vsum[:], vsum_d[64 * b:64 * (b + 1), :])
            for j in range(JPI):
                i = JPI * b + j
                rs = slice(TT * i, TT * (i + 1))
                xt1 = xinp.tile([TT, HID], BF16, tag="x0")
                nc.sync.dma_start(xt1[:], x_d[rs, :])
                # roped alpha*img, feature-major [128, 8 chunks x 112]
                itT = itTp.tile([128, 8 * TT], BF16, tag="itT")
                nc.sync.dma_start(itT[:], imgT_d[128 * i:128 * (i + 1), :])
                # x transposes: 8 bf16 chunks fit one psum bank -> xts [128, 896]
                xts = xtsp.tile([128, 8 * TT], BF16, tag="xts")
                tp = ptrp.tile([128, 8 * TT], BF16, tag="tr")
                for k in range(8):
                    nc.tensor.transpose(tp[:, TT * k:TT * (k + 1)],
                                        xt1[:, 128 * k:128 * (k + 1)],
                                        idn[0:TT, 0:TT])
                nc.scalar.activation(xts[:], tp[:], ACTF.Copy)
                # qk projection, 4 chunks of 512
                for n in range(4):
                    pq = pqp.tile([TT, 512], F32, tag="pq")
                    for k in range(8):
                        nc.tensor.matmul(pq[:],
                                         xts[:, TT * k:TT * (k + 1)],
                                         wqk_sb[k][:, 512 * n:512 * (n + 1)],
                                         start=(k == 0), stop=(k == 7))
                    qkc = qkcp.tile([TT, 512], BF16, tag="qkc")
                    nc.vector.tensor_copy(qkc[:], pq[:])
                    # rope on width-halves (8 heads per chunk)
                    hh = 8 * (n % 2)
                    qv = qkc[:].rearrange("p (h d) -> p h d", d=64)[:, :, 32:64]
                    cv = ct[:].rearrange("p (h d) -> p h d", d=32)[:, hh:hh + 8, :]
                    sv = st[:].rearrange("p (h d) -> p h d", d=32)[:, hh:hh + 8, :]
                    t1 = rtp.tile([TT, 256], BF16, tag="t1")
                    t1v = t1[:].rearrange("p (h d) -> p h d", d=32)
                    t2 = rtp.tile([TT, 256], BF16, tag="t2")
                    t2v = t2[:].rearrange("p (h d) -> p h d", d=32)
                    nc.vector.tensor_tensor(t1v[:], qv[:], cv[:], op=ALU.mult)
                    nc.gpsimd.tensor_tensor(t2v[:, :, 0:16], qv[:, :, 16:32],
                                            sv[:, :, 0:16], op=ALU.mult)
                    nc.gpsimd.tensor_tensor(t2v[:, :, 16:32], qv[:, :, 0:16],
                                            sv[:, :, 16:32], op=ALU.mult)
                    nc.vector.tensor_tensor(qv[:], t1v[:], t2v[:], op=ALU.add)
                    # transpose to feature-major; img (host-roped) added in
                    # the psum->sbuf move
                    tpq = ptrp.tile([128, 8 * TT], BF16, tag="tr",
                                    name=f"tpq{b}_{j}_{n}")
                    for c in range(4):
                        nc.tensor.transpose(tpq[:, TT * c:TT * (c + 1)],
                                            qkc[:, 128 * c:128 * (c + 1)],
                                            idn[0:TT, 0:TT])
                    nc.vector.tensor_tensor(
                        qfkv[:, 4 * n:4 * n + 4, TT * j:TT * (j + 1)],
                        tpq[:, 0:4 * TT].rearrange("p (c t) -> p c t", t=TT),
                        itT[:, 448 * (n % 2):448 * (n % 2 + 1)].rearrange(
                            "p (c t) -> p c t", t=TT),
                        op=ALU.add)

            if stage in ("qfk_q", "qfk_k"):
                if b == 0:
                    off = 0 if stage == "qfk_q" else 8
                    for cg in range(8):
                        ytmp = youtp.tile([128, TPI], F32, tag="y_sb",
                                          name=f"yq{cg}")
                        nc.vector.tensor_copy(ytmp[:], qfkv[:, off + cg, :])
                        nc.sync.dma_start(
                            y_d[128 * cg:128 * (cg + 1), 0:TPI], ytmp[:])
                continue
            if b == 0:
                for k in range(8):
                    t = wop.tile([128, HID], BF16, tag=f"wo{k}",
                                 name=f"wo_sb{k}")
                    nc.sync.dma_start(t[:], wo_d[128 * k:128 * (k + 1), :])
                    wo_sb.append(t)
            elif stage == "full":
                # software pipelining: previous image's output projection is
                # emitted after this image's tile loop so its PE work fills
                # the GroupNorm-epilogue latency
                _outproj(b - 1)

            # ---------------- attention + stats ----------------
            of_all = ofp.tile([128, 8 * TPI], BF16, tag="of", name=f"of{b}")
            of2 = of2p.tile([128, 8 * TPI], BF16, tag="of2", name=f"of2_{b}")
            of2_l[b] = of2
            statb = statp.tile([128, 32], F32, tag="statb")
            nc.gpsimd.memset(statb[:], 0.0)
            sqd = sqdp.tile([128, TPI], BF16, tag="sqd")
            for p in range(8):
                # scores for heads 2p (rows 0:28) and 2p+1 (rows 32:60),
                # 14 rows per psum half
                st_sb = stsbp.tile([64, TPI], BF16, tag="st_sb")
                for half in range(2):
                    stp = stp_ab[half]
                    for hn in range(2):
                        hb = 64 * hn
                        cgq = p
                        cgk = 8 + p
                        for rr in range(14):
                            r = 14 * half + rr
                            nc.tensor.matmul(
                                stp[32 * hn:32 * hn + 28, 32 * rr:32 * rr + 28],
                                qfkv[hb:hb + 64, cgk, 28 * r:28 * (r + 1)],
                                qfkv[hb:hb + 64, cgq, 28 * r:28 * (r + 1)],
                                tile_position=(hb, 32 * hn),
                                start=True, stop=True,
                                skip_group_check=True)
                    stv = stp[:].rearrange("p (r c) -> p r c", c=32)[:, :, 0:28]
                    nc.vector.tensor_copy(
                        st_sb[:, 392 * half:392 * (half + 1)].rearrange(
                            "p (r c) -> p r c", c=28), stv)
                # out = vsum^T @ S^T; statb layout (group-major, g = p//4):
                # sum col 8g + (p%4), sumsq col 8g + 4 + (p%4)
                g, pg = p // 4, p % 4
                ot_ps = potp.tile([128, TPI], F32, tag="ot", name=f"ot{b}_{p}")
                for c0, c1 in ((0, 512), (512, TPI)):
                    for hn in range(2):
                        n = 2 * p + hn
                        hb = 64 * hn
                        rb = 32 * hn
                        nc.tensor.matmul(ot_ps[hb:hb + 64, c0:c1],
                                         vsum[rb:rb + 28, 64 * n:64 * (n + 1)],
                                         st_sb[rb:rb + 28, c0:c1],
                                         tile_position=(rb, hb),
                                         start=True, stop=True,
                                         skip_group_check=True)
                sc = 8 * g + pg
                nc.scalar.activation(
                    of_all[:, TPI * p:TPI * (p + 1)], ot_ps[:],
                    ACTF.Copy, accum_out=statb[:, sc:sc + 1])
                qc = 8 * g + 4 + pg
                nc.scalar.activation(sqd[:], of_all[:, TPI * p:TPI * (p + 1)],
                                     ACTF.Square,
                                     accum_out=statb[:, qc:qc + 1])

                if stage == "of" and b == 0:
                    ytmp = youtp.tile([128, TPI], F32, tag="y_sb",
                                      name=f"yo{p}")
                    nc.vector.tensor_copy(ytmp[:], of_all[:, TPI * p:TPI * (p + 1)])
                    nc.sync.dma_start(y_d[128 * p:128 * (p + 1), 0:TPI],
                                      ytmp[:])
                    continue

                if pg == 3:
                    # ---------------- GroupNorm for heads 8g..8g+7 ----------
                    cs = slice(8 * g, 8 * g + 8)
                    allred = statp.tile([128, 24], F32, tag="allred",
                                        name=f"ar{b}_{g}")
                    # partition_all_reduce only works at partition base 0 on
                    # HW: shift the upper half down, reduce, shift back up
                    sthi = statp.tile([64, 16], F32, tag="sthi",
                                      name=f"sthi{b}_{g}")
                    nc.vector.tensor_copy(sthi[0:64, 0:8], statb[64:128, cs])
                    nc.gpsimd.partition_all_reduce(
                        allred[0:64, 0:8], statb[0:64, cs], channels=64,
                        reduce_op=bass_isa.ReduceOp.add)
                    nc.gpsimd.partition_all_reduce(
                        sthi[0:64, 8:16], sthi[0:64, 0:8], channels=64,
                        reduce_op=bass_isa.ReduceOp.add)
                    nc.vector.tensor_copy(allred[64:128, 0:8],
                                          sthi[0:64, 8:16])
                    stt = statp.tile([128, 24], F32, tag="stt",
                                     name=f"stt{b}_{g}")
                    # cols: 0:4 mean, 4:8 e2, 8:12 msq, 12:16 var, 16:20 sd
                    nc.scalar.mul(stt[:, 0:4], allred[:, 0:4], 1.0 / NGRP)
                    nc.scalar.mul(stt[:, 4:8], allred[:, 4:8], 1.0 / NGRP)
                    nc.scalar.activation(stt[:, 8:12], stt[:, 0:4], ACTF.Square)
                    nc.vector.tensor_tensor(stt[:, 12:16], stt[:, 4:8],
                                            stt[:, 8:12], op=ALU.subtract)
                    nc.scalar.activation(stt[:, 16:20], stt[:, 12:16],
                                         ACTF.Sqrt, bias=epsb[:, 0:1])
                    acs = statp.tile([128, 8], F32, tag="acs",
                                     name=f"acs{b}_{g}")
                    nc.vector.reciprocal(acs[:, 0:4], stt[:, 16:20])
                    nc.vector.tensor_tensor(acs[:, 0:4], acs[:, 0:4],
                                            gw2[:, 4 * g:4 * g + 4],
                                            op=ALU.mult)
                    nc.vector.scalar_tensor_tensor(acs[:, 4:8], stt[:, 0:4],
                                                   -1.0, acs[:, 0:4],
                                                   ALU.mult, ALU.mult)
                    nc.vector.tensor_tensor(acs[:, 4:8], acs[:, 4:8],
                                            gb2[:, 4 * g:4 * g + 4],
                                            op=ALU.add)
                    if stage == "stats" and b == 0:
                        ytmp = youtp.tile([128, 64], F32, tag="y_sb",
                                          name=f"ys{g}")
                        nc.vector.tensor_copy(ytmp[:, 0:32], statb[:])
                        nc.vector.tensor_copy(ytmp[:, 32:56], allred[:])
                        nc.vector.tensor_copy(ytmp[:, 56:64], acs[:])
                        nc.sync.dma_start(
                            y_d[128 * g:128 * (g + 1), 0:64], ytmp[:])
                    for pp in range(4 * g, 4 * g + 4):
                        sca = acs[:, pp % 4:pp % 4 + 1]
                        bia = acs[:, 4 + pp % 4:5 + pp % 4]
                        if pp % 4 < 2:
                            nc.scalar.activation(of2[:, TPI * pp:TPI * (pp + 1)],
                                                 of_all[:, TPI * pp:TPI * (pp + 1)],
                                                 ACTF.Identity,
                                                 scale=sca, bias=bia)
                        else:
                            nc.vector.tensor_scalar(
                                of2[:, TPI * pp:TPI * (pp + 1)],
                                of_all[:, TPI * pp:TPI * (pp + 1)],
                                sca, bia, ALU.mult, ALU.add)

        if stage == "full":
            _outproj(B_CORE - 1)
    nc.compile()
    return nc


def _host_tables():
    from ml_dtypes import bfloat16
    inv_freq = 1.0 / (10000.0 ** (np.arange(0, 16, dtype=np.float64) * 2 / 32))
    wpos = np.arange(W, dtype=np.float64)
    ang = wpos[:, None] * inv_freq[None, :]          # [28, 16]
    cosw = np.cos(ang).astype(np.float32)
    sinw = np.sin(ang).astype(np.float32)
    cblk = np.concatenate([cosw, cosw], axis=1)       # [28, 32]
    sblk = np.concatenate([-sinw, sinw], axis=1)      # [28, 32]
    crow = np.tile(cblk, (1, HEADS))                  # [28, 512]
    srow = np.tile(sblk, (1, HEADS))
    ctab = np.tile(crow, (4, 1)).astype(bfloat16)     # [112, 512]
    stab = np.tile(srow, (4, 1)).astype(bfloat16)
    idn = np.eye(128, dtype=bfloat16)
    return ctab, stab, idn, cosw, sinw


def _make_in_maps(x, input_img, qkv_w, o_w):
    from ml_dtypes import bfloat16
    ctab, stab, idn, cosw, sinw = _host_tables()
    x_bf = np.ascontiguousarray(x).astype(bfloat16)
    wqk = np.ascontiguousarray(
        np.concatenate([qkv_w[:, 0:HID], qkv_w[:, 2 * HID:3 * HID]],
                       axis=1)).astype(bfloat16)
    wo = np.ascontiguousarray(o_w).astype(bfloat16)

    # host vsum: (A@x) @ Wv + alpha*(A@img), duplicated at rows 0:28/32:60
    xsum = x.sum(axis=1).reshape(B_FULL * W, HID)          # [b*28, 1024]
    isum = input_img.sum(axis=1).reshape(B_FULL * W, HID)
    vs = (xsum @ qkv_w[:, HID:2 * HID] + ALPHA * isum).reshape(B_FULL, W, HID)
    vsum_all = np.zeros((B_FULL, 64, HID), dtype=bfloat16)
    vsum_all[:, 0:28] = vs.astype(bfloat16)
    vsum_all[:, 32:60] = vsum_all[:, 0:28]

    # host-roped alpha*img, feature-major per core
    ia = (input_img.reshape(B_FULL, H, W, HEADS, DH) * ALPHA).astype(np.float32)
    th = ia[..., 32:48]
    tw = ia[..., 48:64]
    c = cosw[None, None, :, None, :]                       # [1,1,28,1,16]
    s = sinw[None, None, :, None, :]
    rw_lo = th * c - tw * s
    rw_hi = tw * c + th * s
    ir = np.concatenate([ia[..., 0:32], rw_lo, rw_hi], axis=-1)
    ir = ir.reshape(B_FULL, TPI, HID)

    in_maps = []
    for cix in range(N_CORES):
        sl = slice(B_CORE * cix, B_CORE * (cix + 1))
        # [28 tiles][128 part][8 chunks x 112 tok]
        imgT = np.ascontiguousarray(
            ir[sl].reshape(NTILES, TT, 8, 128).transpose(0, 3, 2, 1)
            .reshape(NTILES * 128, 8 * TT)).astype(bfloat16)
        in_maps.append({
            "x": x_bf[sl].reshape(TOK, HID),
            "imgT": imgT,
            "vsum": vsum_all[sl].reshape(B_CORE * 64, HID),
            "wqk": wqk, "wo": wo,
            "idn": idn, "ctab": ctab, "stab": stab,
        })
    return in_maps


def kernel(x, input_img, qkv_w, o_w, gn_w, gn_b):
    x = np.asarray(x, dtype=np.float32)
    input_img = np.asarray(input_img, dtype=np.float32)
    qkv_w = np.asarray(qkv_w, dtype=np.float32)
    o_w = np.asarray(o_w, dtype=np.float32)
    gn_w = np.asarray(gn_w, dtype=np.float32)
    gn_b = np.asarray(gn_b, dtype=np.float32)

    key = (tuple(gn_w.tolist()), tuple(gn_b.tolist()))
    if key not in _CACHE:
        _CACHE[key] = _build_program(gn_w, gn_b)
    nc = _CACHE[key]

    in_maps = _make_in_maps(x, input_img, qkv_w, o_w)
    res = run_bass_kernel_spmd(nc, in_maps, list(range(N_CORES)))
    out = np.concatenate(
        [res.results[c]["y"].reshape(B_CORE, H, W, HID)
         for c in range(N_CORES)], axis=0)
    return out
